# revision 2
# baseline (speedup 1.0000x reference)
"""Fused BiLSTM-CRF kernel, v2: single NEFF, 8-way batch-parallel.

Each core owns 16 sequences and runs BOTH LSTM directions as two
interleaved dependence chains, then the CRF for the same sequences —
no host round-trip between phases.

LSTM dataflow (per chain): weight-stationary "orientation B".  The
recurrent matmul keeps W_hh chunks stationary in the PE array and
streams the tiny h state, so the pre-activation gates land in PSUM as
[128 gate-dims, 8 chunks x 16 seqs] — full 128-partition tiles for
every elementwise op, and h is produced already transposed for the
next step's matmul (no PE transposes, no copies).

The x-projection (W_ih x + b) is a bulk GEMM streamed straight into
the same PSUM gate banks 4 steps ahead of the recurrence (identity/
copy ops never touch it), with the bias added via K=1 rank-1 matmuls.

Gate math per step (4 DVE stt + 2 ACT tanh):
  T = tanh(G)            G holds [i|f|o|g] with i,f,o rows pre-halved
  A = (T[i|f]+1) * [T_g | C2]        C2 state = 2c, bf16
  C2 = 0.5*A_f + A_ig
  TH = tanh(0.5*C2)                  (activation input scale)
  H = (T_o+1) * TH  = 2h             (W_hh, W_out pre-halved on host)

CRF: exp-space partition function split into an alpha-forward chain
and a beta-backward chain that run concurrently and meet in the
middle (Z = sum_i alpha_K[i] beta_K[i]); rescale every 16 steps with
the ln() of the scale sums deferred to one batched pass.  Gold-path
emission sum via one-hot multiply with accum_out.  Note: this
toolchain's GPSIMD only supports memset-class ops and cannot touch
PSUM, so all elementwise compute sits on DVE/ACT.
"""
import numpy as np
import ml_dtypes

import bass_rust
import concourse.bass as bass
import concourse.tile as tile
from concourse import mybir
from concourse.bass_utils import run_bass_kernel_spmd
from bass_rust import ScopedClock

f32 = mybir.dt.float32
bf16 = mybir.dt.bfloat16
P = 128

V, E, H, T = 50000, 256, 256, 50
B, S = 128, 512
NS = 16          # sequences per core
NCORES = 8
GRP = 4          # steps per PSUM gate-bank group
ERP = 8          # steps per emission batch (H ring size)
bfnp = ml_dtypes.bfloat16

AF = mybir.ActivationFunctionType
OP = mybir.AluOpType


class _TC(tile.TileContext):
    """Split multi-sem waits into NoOp prefixes (toolchain walrus limit)."""

    def _split_waits(self, inst):
        si = getattr(inst, "sync_info", None)
        if si is None or not si.on_wait or len(si.on_wait) <= 1:
            return []
        if inst.engine == mybir.EngineType.Unassigned:
            return []
        waits = list(si.on_wait)
        si.on_wait = waits[-1:]
        nops = []
        for w in waits[:-1]:
            nops.append(bass_rust.InstNoOp(
                text_hint="wsplit", bass_nofuse=True,
                name=self.nc.get_next_instruction_name(),
                engine=inst.engine, ins=[], outs=[],
                sync_info=mybir.SyncInfo(on_wait=[w], on_update=[]),
            ))
        return nops

    def _add_instruction(self, inst):
        for n in self._split_waits(inst):
            super()._add_instruction(n)
        super()._add_instruction(inst)

    def _drain_and_barrier(self, tick_clock, wait_clock):
        nc = self.nc
        probe = nc.sync.nop(hint="tail_wait_probe", nofuse=True)
        wait_clock.add_sem_waits(probe.ins,
                                ScopedClock({None: tick_clock.global_clock}))
        si = probe.ins.sync_info
        waits = list(si.on_wait) if si is not None else []
        if si is not None:
            si.on_wait = waits[:1]
        for w in waits[1:]:
            n2 = nc.sync.nop(hint="tail_wait", nofuse=True)
            si2 = n2.ins.sync_info
            if si2 is None:
                n2.ins.sync_info = mybir.SyncInfo(on_wait=[w], on_update=[])
            else:
                si2.on_wait = [w]
        nc.sync.drain()
        nc.all_engine_barrier()
        popped = nc._tile_sem_poison_stack.pop()
        assert popped is self._sem_poison
        nc.clear_and_free_semaphores(list(self.sems.allocated().values()))
        nc.all_engine_barrier()


def _ln_any(nc, sc, out_ap, in_ps, w, name):
    """ln(x) for large positive x via frexp + ACT Ln on the mantissa."""
    i32 = mybir.dt.int32
    bits = sc.tile([1, w], i32, tag="lnbits", name=f"bits_{name}")
    nc.vector.tensor_copy(bits[:].bitcast(f32), in_ps)
    eint = sc.tile([1, w], i32, tag="lne", name=f"e_{name}")
    nc.vector.tensor_scalar(eint[:], bits[:], 23, None,
                            op0=OP.arith_shift_right)
    ef = sc.tile([1, w], f32, tag="lnef", name=f"ef_{name}")
    nc.vector.tensor_copy(ef[:], eint[:])
    mant = sc.tile([1, w], i32, tag="lnm", name=f"m_{name}")
    nc.vector.tensor_scalar(mant[:], bits[:], 0x007FFFFF, 0x3F800000,
                            op0=OP.bitwise_and, op1=OP.bitwise_or)
    lm = sc.tile([1, w], f32, tag="lnlm", name=f"lm_{name}")
    nc.scalar.activation(lm[:], mant[:].bitcast(f32), AF.Ln)
    ln2 = 0.6931471805599453
    t = sc.tile([1, w], f32, tag="lnt", name=f"t_{name}")
    nc.vector.tensor_scalar(t[:], ef[:], ln2, -127.0 * ln2,
                            op0=OP.mult, op1=OP.add)
    nc.vector.tensor_tensor(out_ap, lm[:], t[:], op=OP.add)


def build_fused(steps=S, debug_emis=False):
    nc = bass.Bass()
    SB = steps * NS
    n_grp = steps // GRP
    if debug_emis:
        dbg_d = nc.declare_dram_parameter("dbg", [T, SB], f32, isOutput=True)
        dbgf_d = nc.declare_dram_parameter("dbgf", [T, SB], f32, isOutput=True)
        dbgb_d = nc.declare_dram_parameter("dbgb", [T, SB], f32, isOutput=True)
        dbgr_d = nc.declare_dram_parameter("dbgr", [P, ERP, 2, NS], f32,
                                           isOutput=True)
        dbgu_d = nc.declare_dram_parameter("dbgu", [P, 160], f32, isOutput=True)
        dbgg_d = nc.declare_dram_parameter("dbgg", [P, P], f32, isOutput=True)

    xT_d = nc.declare_dram_parameter("xT", [P, 2, SB], bf16, isOutput=False)
    wih_d = nc.declare_dram_parameter("wih", [P, 2, 2, 8, P], bf16, isOutput=False)
    whh_d = nc.declare_dram_parameter("whh", [P, 2, 2, 8, P], bf16, isOutput=False)
    bias_d = nc.declare_dram_parameter("biasT", [1, 2, 1024], bf16, isOutput=False)
    wout_d = nc.declare_dram_parameter("woutT", [P, 2, 2, T], bf16, isOutput=False)
    etr_d = nc.declare_dram_parameter("etrans", [T, T], f32, isOutput=False)
    etrB_d = nc.declare_dram_parameter("etransB", [T, T], f32, isOutput=False)
    bout_d = nc.declare_dram_parameter("bout", [T, 1], f32, isOutput=False)
    estart_d = nc.declare_dram_parameter("estart", [T, 1], f32, isOutput=False)
    eend_d = nc.declare_dram_parameter("eend", [T, 1], f32, isOutput=False)
    oh_d = nc.declare_dram_parameter("oh", [T, SB], bf16, isOutput=False)
    out_d = nc.declare_dram_parameter("out", [1, 2 * NS], f32, isOutput=True)

    with _TC(nc) as tc:
        with (
            tc.tile_pool(name="cst", bufs=1) as cp,
            tc.tile_pool(name="big", bufs=1) as bigp,
            tc.tile_pool(name="sc", bufs=3) as sc,
            tc.tile_pool(name="nsc", bufs=2) as nsc,
            tc.tile_pool(name="g0", bufs=2, space="PSUM") as gp0,
            tc.tile_pool(name="g1", bufs=2, space="PSUM") as gp1,
            tc.tile_pool(name="psE", bufs=1, space="PSUM") as psE_p,
            tc.tile_pool(name="psC", bufs=3, space="PSUM") as psC,
        ):
            gpool = [gp0, gp1]
            # ---- constants / weights ----
            wih = cp.tile([P, 2, 2, 8, P], bf16, name="wih")
            whh = cp.tile([P, 2, 2, 8, P], bf16, name="whh")
            biasT = cp.tile([1, 2, 1024], bf16, name="biasT")
            woutT = cp.tile([P, 2, 2, T], bf16, name="woutT")
            etr = cp.tile([T, T], f32, name="etr")
            etrB = cp.tile([T, T], f32, name="etrB")
            nc.sync.dma_start(etrB[:], etrB_d[:])
            bout = cp.tile([T, 1], f32, name="bout")
            estart = cp.tile([T, 1], f32, name="estart")
            eend = cp.tile([T, 1], f32, name="eend")
            nc.sync.dma_start(wih[:], wih_d[:])
            nc.sync.dma_start(whh[:], whh_d[:])
            nc.sync.dma_start(biasT[:], bias_d[:])
            nc.sync.dma_start(woutT[:], wout_d[:])
            nc.sync.dma_start(etr[:], etr_d[:])
            nc.sync.dma_start(bout[:], bout_d[:])
            nc.sync.dma_start(estart[:], estart_d[:])
            nc.sync.dma_start(eend[:], eend_d[:])

            ones64 = cp.tile([1, 64], bf16, name="ones64")
            ones50c = cp.tile([T, 1], f32, name="ones50c")
            ones50r = cp.tile([1, T], f32, name="ones50r")
            nc.gpsimd.memset(ones64[:], 1.0)
            nc.gpsimd.memset(ones50c[:], 1.0)
            nc.gpsimd.memset(ones50r[:], 1.0)

            xT = bigp.tile([P, 2, SB], bf16, name="xT")
            nx = max(1, min(8, SB // 1024))
            xc = SB // nx
            for i in range(nx):
                nc.sync.dma_start(xT[:, :, i * xc:(i + 1) * xc],
                                  xT_d[:, :, i * xc:(i + 1) * xc])
            oh = bigp.tile([T, SB], bf16, name="oh")
            nc.sync.dma_start(oh[:], oh_d[:])

            # ---- per-chain state ----
            U = [cp.tile([P, 160], bf16, name=f"U{d}") for d in (0, 1)]
            A = [cp.tile([P, 64], bf16, name=f"A{d}") for d in (0, 1)]
            TH = [cp.tile([P, 32], bf16, name=f"TH{d}") for d in (0, 1)]
            R = [cp.tile([P, ERP, 2, NS], bf16, name=f"R{d}") for d in (0, 1)]
            Z = cp.tile([P, 2, NS], bf16, name="Z")
            nc.gpsimd.memset(Z[:], 0.0)
            for d in (0, 1):
                nc.gpsimd.memset(U[d][:, 128:160], 0.0)

            emis = [bigp.tile([T, SB], bf16, name=f"emis{d}") for d in (0, 1)]

            # slot index for step s of chain d (bwd ring is time-flipped so
            # emission batches come out in original time order)
            def slot(d, s):
                return (s % ERP) if d == 0 else (ERP - 1 - s % ERP)

            # original-time column base of the gate group / emission batch
            def gcol(d, g):
                return 64 * g if d == 0 else SB - 64 * (g + 1)

            def ecol(d, s8):
                return ERP * NS * s8 if d == 0 else SB - ERP * NS * (s8 + 1)

            G_cur = [None, None]
            G_nxt = [None, None]

            def make_ih_jobs(d, g):
                gt = gpool[d].tile([P, GRP * P], f32, tag=f"G{d}",
                                   name=f"G{d}_{g}")
                jobs = []
                c0 = gcol(d, g)
                for m in range(8):
                    for k in (0, 1):
                        # start=True clears the WHOLE 2KB psum bank, so only
                        # the very first matmul into this tile may set it.
                        def j(d=d, m=m, k=k, gt=gt, c0=c0):
                            nc.tensor.matmul(
                                gt[:, m * 64:(m + 1) * 64],
                                wih[:, d, k, m, :],
                                xT[:, k, c0:c0 + 64],
                                start=(m == 0 and k == 0), stop=False,
                                skip_group_check=True)
                        jobs.append(j)
                    def jb(d=d, m=m, gt=gt):
                        nc.tensor.matmul(
                            gt[:, m * 64:(m + 1) * 64],
                            biasT[:, d, m * P:(m + 1) * P],
                            ones64[:, :],
                            start=False, stop=False, skip_group_check=True)
                    jobs.append(jb)
                return gt, jobs

            for d in (0, 1):
                G_cur[d], jobs = make_ih_jobs(d, 0)
                for j in jobs:
                    j()
            pend = [[], []]
            if n_grp > 1:
                for d in (0, 1):
                    G_nxt[d], pend[d] = make_ih_jobs(d, 1)

            psE = [None, None]
            for s in range(steps):
                s_locs = {}
                for d in (0, 1):
                    g = s // GRP
                    s_loc = (s % GRP) if d == 0 else (GRP - 1 - s % GRP)
                    s_locs[d] = (G_cur[d], s_loc)
                    gt = G_cur[d]
                    rhs = Z if s == 0 else R[d][:, slot(d, s - 1), :, :]
                    for m in range(8):
                        for k in (0, 1):
                            nc.tensor.matmul(
                                gt[:, m * 64 + s_loc * NS:
                                   m * 64 + (s_loc + 1) * NS],
                                whh[:, d, k, m, :],
                                rhs[:, k, :],
                                start=False, stop=(k == 1),
                                skip_group_check=True)

                    # spread next group's ih matmuls over this group's steps
                    n_do = (len(pend[d]) + GRP - 1 - s % GRP) // (GRP - s % GRP) \
                        if pend[d] else 0
                    for _ in range(n_do):
                        pend[d].pop(0)()
                    if s % GRP == GRP - 1 and g + 2 < n_grp:
                        assert not pend[d]
                        G_nxt_new, pend[d] = make_ih_jobs(d, g + 2)
                        G_cur[d] = G_nxt[d]
                        G_nxt[d] = G_nxt_new
                    elif s % GRP == GRP - 1:
                        G_cur[d] = G_nxt[d]

                for d in (0, 1):
                    gt, s_loc = s_locs[d]
                    gview = gt[:].rearrange("p (m sb) -> p m sb", m=8)[
                        :, :, s_loc * NS:(s_loc + 1) * NS]
                    uview = U[d][:, 0:128].rearrange("p (m b) -> p m b", m=8)
                    if debug_emis and s == 0 and d == 0:
                        gg = bigp.tile([P, P], f32, name="gg")
                        nc.vector.tensor_copy(
                            gg[:].rearrange("p (m b) -> p m b", m=8), gview)
                        nc.sync.dma_start(dbgg_d[:], gg[:])
                    nc.scalar.activation(uview, gview, AF.Tanh)

                for d in (0, 1):
                    nc.vector.scalar_tensor_tensor(
                        A[d][:], U[d][:, 0:64], 1.0, U[d][:, 96:160],
                        op0=OP.add, op1=OP.mult)
                    nc.vector.scalar_tensor_tensor(
                        U[d][:, 128:160], A[d][:, 32:64], 0.5, A[d][:, 0:32],
                        op0=OP.mult, op1=OP.add)
                for d in (0, 1):
                    nc.scalar.activation(TH[d][:], U[d][:, 128:160], AF.Tanh,
                                         scale=0.5)
                for d in (0, 1):
                    nc.vector.scalar_tensor_tensor(
                        R[d][:, slot(d, s), :, :], U[d][:, 64:96], 1.0,
                        TH[d][:], op0=OP.add, op1=OP.mult)
                    if debug_emis and s == 0 and d == 0:
                        uu = bigp.tile([P, 160], f32, name="uu")
                        nc.vector.tensor_copy(uu[:], U[d][:])
                        nc.sync.dma_start(dbgu_d[:], uu[:])

                if s % ERP == ERP - 1:
                    for d in (0, 1):
                        pe = psE_p.tile([T, ERP * NS], f32, tag="psE",
                                        name=f"psE{d}_{s}")
                        for k in (0, 1):
                            nc.tensor.matmul(
                                pe[:].rearrange("p (sl b) -> p sl b", sl=ERP),
                                woutT[:, d, k, :],
                                R[d][:, :, k, :],
                                start=(k == 0), stop=(k == 1),
                                skip_group_check=True)
                        # Pool/GPSIMD cannot read PSUM on HW: copies go on DVE
                        c0 = ecol(d, s // ERP)
                        nc.vector.tensor_copy(
                            emis[d][:, c0:c0 + ERP * NS], pe[:])

            # ---- emissions -> eexp, numerator ----
            esum = bigp.tile([T, SB], f32, name="esum")
            nc.vector.tensor_tensor(esum[:], emis[0][:], emis[1][:], op=OP.add)
            if debug_emis:
                nc.sync.dma_start(dbg_d[:], esum[:])
                ef32 = bigp.tile([T, SB], f32, name="ef32")
                nc.vector.tensor_copy(ef32[:], emis[0][:])
                nc.sync.dma_start(dbgf_d[:], ef32[:])
                eb32 = bigp.tile([T, SB], f32, name="eb32")
                nc.vector.tensor_copy(eb32[:], emis[1][:])
                nc.sync.dma_start(dbgb_d[:], eb32[:])
                r32 = bigp.tile([P, ERP, 2, NS], f32, name="r32")
                nc.vector.tensor_copy(r32[:], R[0][:])
                nc.sync.dma_start(dbgr_d[:], r32[:])
            eexp = bigp.tile([T, SB], f32, name="eexp")
            nc.scalar.activation(eexp[:], esum[:], AF.Exp, bias=bout[:, 0:1])

            out_sb = cp.tile([1, 2 * NS], f32, name="out_sb")
            acc = cp.tile([T, NS], f32, name="acc")
            for b in range(NS):
                scr = nsc.tile([T, steps], f32, tag="numscr", name=f"nscr{b}")
                ev = esum[:].rearrange("p (s b) -> p b s", b=NS)[:, b, :]
                ov = oh[:].rearrange("p (s b) -> p b s", b=NS)[:, b, :]
                nc.vector.scalar_tensor_tensor(
                    scr[:], ev, 1.0, ov, op0=OP.mult, op1=OP.mult,
                    accum_out=acc[:, b:b + 1])
            pnt = psE_p.tile([T, ERP * NS], f32, tag="psE", name="pnumt")
            pnum = pnt[0:1, 0:NS]
            nc.tensor.matmul(pnum, ones50c[:], acc[:], start=True, stop=True,
                             skip_group_check=True)
            nc.vector.tensor_copy(out_sb[0:1, 0:NS], pnum)

            # ---- CRF partition function: alpha-forward and beta-backward
            # halves run concurrently and meet in the middle:
            #   Z = sum_i alpha_K[i] * beta_K[i],  K = steps//2 - 1.
            # One full-width [50,16] multiply per chain-step keeps DVE
            # fixed costs down; rescale sums are stashed and ln()'d in one
            # deferred batched pass.
            RESC = 16
            KMID = steps // 2 - 1
            n_resc = steps // RESC + 2
            shist = [cp.tile([1, n_resc * NS], f32, name=f"shist{c}")
                     for c in (0, 1)]
            nresc_done = [0, 0]

            def rescale(c, cur, j):
                pst = psC.tile([T, NS], f32, tag="pa", name=f"pss{c}_{j}")
                ps_s = pst[0:1, :]
                nc.tensor.matmul(ps_s, ones50c[:], cur[:],
                                 start=True, stop=True, skip_group_check=True)
                rinv = sc.tile([1, NS], f32, tag=f"ri{c}", name=f"ri{c}_{j}")
                nc.vector.reciprocal(rinv[:], ps_s)
                ri = nresc_done[c]
                nc.vector.tensor_copy(shist[c][:, ri * NS:(ri + 1) * NS], ps_s)
                nresc_done[c] += 1
                pb = psC.tile([T, NS], f32, tag="pa", name=f"pb{c}_{j}")
                nc.tensor.matmul(pb[:], ones50r[:], rinv[:],
                                 start=True, stop=True, skip_group_check=True)
                ar = sc.tile([T, NS], f32, tag=f"a{c}", name=f"ar{c}_{j}")
                nc.vector.tensor_tensor(ar[:], pb[:], cur[:], op=OP.mult)
                return ar

            # alpha chain state (c=0), beta chain state y = e_k * beta_k (c=1)
            a_cur = sc.tile([T, NS], f32, tag="a0", name="alf0")
            nc.vector.tensor_scalar(
                out=a_cur[:], in0=eexp[:, 0:NS],
                scalar1=estart[:, 0:1], scalar2=None, op0=OP.mult)
            y_cur = sc.tile([T, NS], f32, tag="a1", name="bet0")
            nc.vector.tensor_scalar(
                out=y_cur[:], in0=eexp[:, (steps - 1) * NS:steps * NS],
                scalar1=eend[:, 0:1], scalar2=None, op0=OP.mult)

            ka = list(range(1, KMID + 1))          # alpha: absorb e_k
            kb = list(range(steps - 2, KMID, -1))  # beta: MM then absorb e_k
            b_fin = None
            for j in range(max(len(ka), len(kb) + 1)):
                if j < len(ka):
                    k = ka[j]
                    pa = psC.tile([T, NS], f32, tag="pa", name=f"paA_{k}")
                    nc.tensor.matmul(pa[:], etr[:], a_cur[:],
                                     start=True, stop=True,
                                     skip_group_check=True)
                    an = sc.tile([T, NS], f32, tag="a0", name=f"alf{k}")
                    nc.vector.tensor_tensor(
                        an[:], pa[:], eexp[:, k * NS:(k + 1) * NS],
                        op=OP.mult)
                    a_cur = an
                    if j % RESC == 14:
                        a_cur = rescale(0, a_cur, j)
                if j < len(kb):
                    k = kb[j]
                    pb_ = psC.tile([T, NS], f32, tag="pa", name=f"pbB_{k}")
                    nc.tensor.matmul(pb_[:], etrB[:], y_cur[:],
                                     start=True, stop=True,
                                     skip_group_check=True)
                    yn = sc.tile([T, NS], f32, tag="a1", name=f"bet{k}")
                    nc.vector.tensor_tensor(
                        yn[:], pb_[:], eexp[:, k * NS:(k + 1) * NS],
                        op=OP.mult)
                    y_cur = yn
                    if j % RESC == 6:
                        y_cur = rescale(1, y_cur, j)
                elif j == len(kb):
                    # last beta MM: beta_KMID = etr @ y_{KMID+1}
                    bf = psC.tile([T, NS], f32, tag="pa", name="betfin")
                    nc.tensor.matmul(bf[:], etrB[:], y_cur[:],
                                     start=True, stop=True,
                                     skip_group_check=True)
                    b_fin = bf

            w = sc.tile([T, NS], f32, tag="a0", name="w_ab")
            nc.vector.tensor_tensor(w[:], a_cur[:], b_fin[:], op=OP.mult)
            pzt = psC.tile([T, NS], f32, tag="pa", name="pzf")
            pz = pzt[0:1, :]
            nc.tensor.matmul(pz, ones50c[:], w[:],
                             start=True, stop=True, skip_group_check=True)
            lz = sc.tile([1, NS], f32, tag="lz", name="lzf")
            _ln_any(nc, sc, lz[:], pz, NS, "fin")
            lsum = sc.tile([1, NS], f32, tag="lsum", name="lsum")
            nc.vector.tensor_copy(lsum[:], lz[:])
            for c in (0, 1):
                nr = nresc_done[c]
                lnh = sc.tile([1, nr * NS], f32, tag="lnh", name=f"lnh{c}")
                _ln_any(nc, sc, lnh[:], shist[c][:, 0:nr * NS], nr * NS,
                        f"hist{c}")
                lacc = sc.tile([1, NS, 1], f32, tag="laccr", name=f"laccr{c}")
                nc.vector.tensor_reduce(
                    lacc[:],
                    lnh[:].rearrange("o (r b) -> o b r", b=NS),
                    axis=mybir.AxisListType.X, op=OP.add)
                nc.vector.tensor_tensor(lsum[:], lsum[:], lacc[:, :, 0],
                                        op=OP.add)
            nc.vector.tensor_copy(out_sb[0:1, NS:2 * NS], lsum[:])

            nc.sync.dma_start(out_d[:], out_sb[:])
    return nc


# ---------------------------------------------------------------------------
# Host-side preparation
# ---------------------------------------------------------------------------
# device gate-chunk order: i0 i1 f0 f1 o0 o1 g0 g1 (torch rows i,f,g,o)
_M_ROWS = [0, 1, 2, 3, 6, 7, 4, 5]   # torch 128-row chunk for device chunk m
_M_SCALE = [.5, .5, .5, .5, .5, .5, 1., 1.]


def _prep_dir_weights(w_ih, w_hh, b_ih, b_hh, w_out_half):
    """Returns (wih [128,2,8,128], whh [128,2,8,128], bias [1024],
    wout [128,2,50]) with all scalings applied."""
    wih_c = np.zeros((P, 2, 8, P), dtype=bfnp)
    whh_c = np.zeros((P, 2, 8, P), dtype=bfnp)
    bias = np.zeros(1024, dtype=np.float32)
    bsum = b_ih + b_hh
    for m in range(8):
        r0 = _M_ROWS[m] * P
        sc_m = _M_SCALE[m]
        for k in range(2):
            wih_c[:, k, m, :] = (w_ih[r0:r0 + P, k * P:(k + 1) * P].T
                                 * sc_m).astype(bfnp)
            # extra 0.5: the device h state carries 2h
            whh_c[:, k, m, :] = (w_hh[r0:r0 + P, k * P:(k + 1) * P].T
                                 * sc_m * 0.5).astype(bfnp)
        bias[m * P:(m + 1) * P] = bsum[r0:r0 + P] * sc_m
    wout_c = np.zeros((P, 2, T), dtype=bfnp)
    for k in range(2):
        wout_c[:, k, :] = (w_out_half[:, k * P:(k + 1) * P].T * 0.5).astype(bfnp)
    return wih_c, whh_c, bias.astype(bfnp), wout_c


def prep_inputs(inputs, steps=S):
    sent = np.asarray(inputs["sentences"])[:, :steps]
    tags = np.asarray(inputs["tags"])[:, :steps]
    emb = np.asarray(inputs["embedding"], dtype=np.float32)
    W_out = np.asarray(inputs["W_out"], dtype=np.float32)

    wihs, whhs, biases, wouts = [], [], [], []
    for d, sfx in enumerate("fb"):
        wi, wh, bs, wo = _prep_dir_weights(
            np.asarray(inputs[f"w_ih_{sfx}"], dtype=np.float32),
            np.asarray(inputs[f"w_hh_{sfx}"], dtype=np.float32),
            np.asarray(inputs[f"b_ih_{sfx}"], dtype=np.float32),
            np.asarray(inputs[f"b_hh_{sfx}"], dtype=np.float32),
            W_out[:, d * H:(d + 1) * H])
        wihs.append(wi); whhs.append(wh); biases.append(bs); wouts.append(wo)
    wih_all = np.ascontiguousarray(np.stack(wihs, axis=1))      # [128,2,2,8,128]
    whh_all = np.ascontiguousarray(np.stack(whhs, axis=1))
    bias_all = np.stack(biases, axis=0)[None]                    # [1,2,1024]
    wout_all = np.ascontiguousarray(np.stack(wouts, axis=1))     # [128,2,2,50]

    trans = np.asarray(inputs["trans"], dtype=np.float32)
    etrans = np.exp(trans).astype(np.float32)
    estart = np.exp(np.asarray(inputs["start_trans"],
                               dtype=np.float32)).reshape(T, 1)
    eend = np.exp(np.asarray(inputs["end_trans"],
                             dtype=np.float32)).reshape(T, 1)
    b_out = np.asarray(inputs["b_out"], dtype=np.float32).reshape(T, 1)

    xs = emb[sent].astype(bfnp)                     # [B, steps, E]
    in_maps = []
    for core in range(NCORES):
        b0 = core * NS
        xc = xs[b0:b0 + NS]                          # [NS, steps, E]
        # -> [E(k,p), steps*NS] with col = s*NS + b
        xT = np.ascontiguousarray(
            xc.transpose(2, 1, 0).reshape(2, P, steps * NS)
            .transpose(1, 0, 2))
        tc_ = tags[b0:b0 + NS]                       # [NS, steps]
        ohc = (np.arange(T)[:, None, None]
               == tc_.T[None, :, :]).astype(bfnp)    # [T, steps, NS]
        in_maps.append({
            "xT": xT,
            "wih": wih_all, "whh": whh_all, "biasT": bias_all,
            "woutT": wout_all,
            "etrans": etrans, "etransB": np.ascontiguousarray(etrans.T), "bout": b_out, "estart": estart, "eend": eend,
            "oh": np.ascontiguousarray(ohc.reshape(T, steps * NS)),
        })
    return in_maps


def host_numerator_part(inputs, steps=S):
    tags = np.asarray(inputs["tags"])[:, :steps]
    trans = np.asarray(inputs["trans"], dtype=np.float32)
    b_out = np.asarray(inputs["b_out"], dtype=np.float32)
    start_trans = np.asarray(inputs["start_trans"], dtype=np.float32)
    end_trans = np.asarray(inputs["end_trans"], dtype=np.float32)
    num = start_trans[tags[:, 0]] + end_trans[tags[:, -1]]
    num = num + trans[tags[:, :-1], tags[:, 1:]].sum(axis=1)
    num = num + b_out[tags].sum(axis=1)
    return num.astype(np.float32)


_cache = {}


def _get_nc(steps):
    if steps not in _cache:
        _cache[steps] = build_fused(steps)
    return _cache[steps]


def run_fused(inputs, steps=S, trace=False):
    nc = _get_nc(steps)
    in_maps = prep_inputs(inputs, steps)
    res = run_bass_kernel_spmd(nc, in_maps, core_ids=list(range(NCORES)),
                               trace=trace)
    num_e = np.concatenate([r["out"][0, 0:NS] for r in res.results])
    logz = np.concatenate([r["out"][0, NS:2 * NS] for r in res.results])
    num_h = host_numerator_part(inputs, steps)
    loss = -np.mean(num_h + num_e - logz)
    return np.float32(loss), {
        "num_e": num_e, "logz": logz, "num_h": num_h,
        "exec": res.exec_time_ns,
    }


def kernel(**inputs):
    loss, _ = run_fused(inputs, steps=S, trace=False)
    return np.asarray(loss, dtype=np.float32)


# revision 3
# speedup vs baseline: 1.0109x; 1.0109x over previous
"""Fused BiLSTM-CRF kernel, v2: single NEFF, 8-way batch-parallel.

Each core owns 16 sequences and runs BOTH LSTM directions as two
interleaved dependence chains, then the CRF for the same sequences —
no host round-trip between phases.

LSTM dataflow (per chain): weight-stationary "orientation B".  The
recurrent matmul keeps W_hh chunks stationary in the PE array and
streams the tiny h state, so the pre-activation gates land in PSUM as
[128 gate-dims, 8 chunks x 16 seqs] — full 128-partition tiles for
every elementwise op, and h is produced already transposed for the
next step's matmul (no PE transposes, no copies).

The x-projection (W_ih x + b) is a bulk GEMM streamed straight into
the same PSUM gate banks 4 steps ahead of the recurrence (identity/
copy ops never touch it), with the bias added via K=1 rank-1 matmuls.

Gate math per step (4 DVE stt + 2 ACT tanh):
  T = tanh(G)            G holds [i|f|o|g] with i,f,o rows pre-halved
  A = (T[i|f]+1) * [T_g | C2]        C2 state = 2c, bf16
  C2 = 0.5*A_f + A_ig
  TH = tanh(0.5*C2)                  (activation input scale)
  H = (T_o+1) * TH  = 2h             (W_hh, W_out pre-halved on host)

CRF: exp-space partition function split into an alpha-forward chain
and a beta-backward chain that run concurrently and meet in the
middle (Z = sum_i alpha_K[i] beta_K[i]); rescale every 16 steps with
the ln() of the scale sums deferred to one batched pass.  Gold-path
emission sum via one-hot multiply with accum_out.  Note: this
toolchain's GPSIMD only supports memset-class ops and cannot touch
PSUM, so all elementwise compute sits on DVE/ACT.
"""
import numpy as np
import ml_dtypes

import bass_rust
import concourse.bass as bass
import concourse.tile as tile
from concourse import mybir
from concourse.bass_utils import run_bass_kernel_spmd
from bass_rust import ScopedClock

f32 = mybir.dt.float32
bf16 = mybir.dt.bfloat16
P = 128

V, E, H, T = 50000, 256, 256, 50
B, S = 128, 512
NS = 16          # sequences per core
NCORES = 8
GRP = 4          # steps per PSUM gate-bank group
ERP = 8          # steps per emission batch (H ring size)
bfnp = ml_dtypes.bfloat16

AF = mybir.ActivationFunctionType
OP = mybir.AluOpType


class _TC(tile.TileContext):
    """Split multi-sem waits into NoOp prefixes (toolchain walrus limit)."""

    def _split_waits(self, inst):
        si = getattr(inst, "sync_info", None)
        if si is None or not si.on_wait or len(si.on_wait) <= 1:
            return []
        if inst.engine == mybir.EngineType.Unassigned:
            return []
        waits = list(si.on_wait)
        si.on_wait = waits[-1:]
        nops = []
        for w in waits[:-1]:
            nops.append(bass_rust.InstNoOp(
                text_hint="wsplit", bass_nofuse=True,
                name=self.nc.get_next_instruction_name(),
                engine=inst.engine, ins=[], outs=[],
                sync_info=mybir.SyncInfo(on_wait=[w], on_update=[]),
            ))
        return nops

    def _add_instruction(self, inst):
        for n in self._split_waits(inst):
            super()._add_instruction(n)
        super()._add_instruction(inst)

    def _drain_and_barrier(self, tick_clock, wait_clock):
        nc = self.nc
        probe = nc.sync.nop(hint="tail_wait_probe", nofuse=True)
        wait_clock.add_sem_waits(probe.ins,
                                ScopedClock({None: tick_clock.global_clock}))
        si = probe.ins.sync_info
        waits = list(si.on_wait) if si is not None else []
        if si is not None:
            si.on_wait = waits[:1]
        for w in waits[1:]:
            n2 = nc.sync.nop(hint="tail_wait", nofuse=True)
            si2 = n2.ins.sync_info
            if si2 is None:
                n2.ins.sync_info = mybir.SyncInfo(on_wait=[w], on_update=[])
            else:
                si2.on_wait = [w]
        nc.sync.drain()
        nc.all_engine_barrier()
        popped = nc._tile_sem_poison_stack.pop()
        assert popped is self._sem_poison
        nc.clear_and_free_semaphores(list(self.sems.allocated().values()))
        nc.all_engine_barrier()


def _ln_any(nc, sc, out_ap, in_ps, w, name):
    """ln(x) for large positive x via frexp + ACT Ln on the mantissa."""
    i32 = mybir.dt.int32
    bits = sc.tile([1, w], i32, tag="lnbits", name=f"bits_{name}")
    nc.vector.tensor_copy(bits[:].bitcast(f32), in_ps)
    eint = sc.tile([1, w], i32, tag="lne", name=f"e_{name}")
    nc.vector.tensor_scalar(eint[:], bits[:], 23, None,
                            op0=OP.arith_shift_right)
    ef = sc.tile([1, w], f32, tag="lnef", name=f"ef_{name}")
    nc.vector.tensor_copy(ef[:], eint[:])
    mant = sc.tile([1, w], i32, tag="lnm", name=f"m_{name}")
    nc.vector.tensor_scalar(mant[:], bits[:], 0x007FFFFF, 0x3F800000,
                            op0=OP.bitwise_and, op1=OP.bitwise_or)
    lm = sc.tile([1, w], f32, tag="lnlm", name=f"lm_{name}")
    nc.scalar.activation(lm[:], mant[:].bitcast(f32), AF.Ln)
    ln2 = 0.6931471805599453
    t = sc.tile([1, w], f32, tag="lnt", name=f"t_{name}")
    nc.vector.tensor_scalar(t[:], ef[:], ln2, -127.0 * ln2,
                            op0=OP.mult, op1=OP.add)
    nc.vector.tensor_tensor(out_ap, lm[:], t[:], op=OP.add)


def build_fused(steps=S, debug_emis=False):
    nc = bass.Bass()
    SB = steps * NS
    n_grp = steps // GRP
    if debug_emis:
        dbg_d = nc.declare_dram_parameter("dbg", [T, SB], f32, isOutput=True)
        dbgr_d = nc.declare_dram_parameter("dbgr", [P, ERP, 2, NS], f32,
                                           isOutput=True)
        dbgu_d = nc.declare_dram_parameter("dbgu", [P, 160], f32, isOutput=True)
        dbgg_d = nc.declare_dram_parameter("dbgg", [P, P], f32, isOutput=True)

    xT_d = nc.declare_dram_parameter("xT", [P, 2, SB], bf16, isOutput=False)
    wih_d = nc.declare_dram_parameter("wih", [P, 2, 2, 8, P], bf16, isOutput=False)
    whh_d = nc.declare_dram_parameter("whh", [P, 2, 2, 8, P], bf16, isOutput=False)
    bias_d = nc.declare_dram_parameter("biasT", [1, 2, 1024], bf16, isOutput=False)
    wout_d = nc.declare_dram_parameter("woutT", [P, 2, 2, T], bf16, isOutput=False)
    etr_d = nc.declare_dram_parameter("etrans", [T, T], f32, isOutput=False)
    etrB_d = nc.declare_dram_parameter("etransB", [T, T], f32, isOutput=False)
    bout_d = nc.declare_dram_parameter("bout", [T, 1], f32, isOutput=False)
    estart_d = nc.declare_dram_parameter("estart", [T, 1], f32, isOutput=False)
    eend_d = nc.declare_dram_parameter("eend", [T, 1], f32, isOutput=False)
    oh_d = nc.declare_dram_parameter("oh", [T, SB], bf16, isOutput=False)
    out_d = nc.declare_dram_parameter("out", [1, 2 * NS], f32, isOutput=True)

    with _TC(nc) as tc:
        with (
            tc.tile_pool(name="cst", bufs=1) as cp,
            tc.tile_pool(name="big", bufs=1) as bigp,
            tc.tile_pool(name="sc", bufs=3) as sc,
            tc.tile_pool(name="nsc", bufs=2) as nsc,
            tc.tile_pool(name="g0", bufs=2, space="PSUM") as gp0,
            tc.tile_pool(name="g1", bufs=2, space="PSUM") as gp1,
            tc.tile_pool(name="psE", bufs=1, space="PSUM") as psE_p,
            tc.tile_pool(name="psC", bufs=3, space="PSUM") as psC,
        ):
            gpool = [gp0, gp1]
            # ---- constants / weights ----
            wih = cp.tile([P, 2, 2, 8, P], bf16, name="wih")
            whh = cp.tile([P, 2, 2, 8, P], bf16, name="whh")
            biasT = cp.tile([1, 2, 1024], bf16, name="biasT")
            woutT = cp.tile([P, 2, 2, T], bf16, name="woutT")
            etr = cp.tile([T, T], f32, name="etr")
            etrB = cp.tile([T, T], f32, name="etrB")
            nc.sync.dma_start(etrB[:], etrB_d[:])
            bout = cp.tile([T, 1], f32, name="bout")
            estart = cp.tile([T, 1], f32, name="estart")
            eend = cp.tile([T, 1], f32, name="eend")
            nc.sync.dma_start(wih[:], wih_d[:])
            nc.sync.dma_start(whh[:], whh_d[:])
            nc.sync.dma_start(biasT[:], bias_d[:])
            nc.sync.dma_start(woutT[:], wout_d[:])
            nc.sync.dma_start(etr[:], etr_d[:])
            nc.sync.dma_start(bout[:], bout_d[:])
            nc.sync.dma_start(estart[:], estart_d[:])
            nc.sync.dma_start(eend[:], eend_d[:])

            ones64 = cp.tile([1, 64], bf16, name="ones64")
            ones50c = cp.tile([T, 1], f32, name="ones50c")
            ones50r = cp.tile([1, T], f32, name="ones50r")
            nc.gpsimd.memset(ones64[:], 1.0)
            nc.gpsimd.memset(ones50c[:], 1.0)
            nc.gpsimd.memset(ones50r[:], 1.0)

            xT = bigp.tile([P, 2, SB], bf16, name="xT")
            nx = max(1, min(8, SB // 1024))
            xc = SB // nx
            for i in range(nx):
                nc.sync.dma_start(xT[:, :, i * xc:(i + 1) * xc],
                                  xT_d[:, :, i * xc:(i + 1) * xc])
            oh = bigp.tile([T, SB], bf16, name="oh")
            nc.sync.dma_start(oh[:], oh_d[:])

            # ---- per-chain state ----
            U = [cp.tile([P, 160], bf16, name=f"U{d}") for d in (0, 1)]
            A = [cp.tile([P, 64], bf16, name=f"A{d}") for d in (0, 1)]
            TH = [cp.tile([P, 32], bf16, name=f"TH{d}") for d in (0, 1)]
            R = [cp.tile([P, ERP, 2, NS], bf16, name=f"R{d}") for d in (0, 1)]
            Z = cp.tile([P, 2, NS], bf16, name="Z")
            nc.gpsimd.memset(Z[:], 0.0)
            for d in (0, 1):
                nc.gpsimd.memset(U[d][:, 128:160], 0.0)

            # both chains accumulate partial emissions directly into esum:
            # for any column range one chain always arrives strictly first
            # (fwd for orig steps < S/2, bwd for the rest), so the early
            # chain copies and the late chain adds.
            esum = bigp.tile([T, SB], f32, name="esum")

            # slot index for step s of chain d (bwd ring is time-flipped so
            # emission batches come out in original time order)
            def slot(d, s):
                return (s % ERP) if d == 0 else (ERP - 1 - s % ERP)

            # original-time column base of the gate group / emission batch
            def gcol(d, g):
                return 64 * g if d == 0 else SB - 64 * (g + 1)

            def ecol(d, s8):
                return ERP * NS * s8 if d == 0 else SB - ERP * NS * (s8 + 1)

            G_cur = [None, None]
            G_nxt = [None, None]

            def make_ih_jobs(d, g):
                gt = gpool[d].tile([P, GRP * P], f32, tag=f"G{d}",
                                   name=f"G{d}_{g}")
                jobs = []
                c0 = gcol(d, g)
                for m in range(8):
                    for k in (0, 1):
                        # start=True clears the WHOLE 2KB psum bank, so only
                        # the very first matmul into this tile may set it.
                        def j(d=d, m=m, k=k, gt=gt, c0=c0):
                            nc.tensor.matmul(
                                gt[:, m * 64:(m + 1) * 64],
                                wih[:, d, k, m, :],
                                xT[:, k, c0:c0 + 64],
                                start=(m == 0 and k == 0), stop=False,
                                skip_group_check=True)
                        jobs.append(j)
                    def jb(d=d, m=m, gt=gt):
                        nc.tensor.matmul(
                            gt[:, m * 64:(m + 1) * 64],
                            biasT[:, d, m * P:(m + 1) * P],
                            ones64[:, :],
                            start=False, stop=False, skip_group_check=True)
                    jobs.append(jb)
                return gt, jobs

            for d in (0, 1):
                G_cur[d], jobs = make_ih_jobs(d, 0)
                for j in jobs:
                    j()
            pend = [[], []]
            if n_grp > 1:
                for d in (0, 1):
                    G_nxt[d], pend[d] = make_ih_jobs(d, 1)

            psE = [None, None]
            for s in range(steps):
                s_locs = {}
                for d in (0, 1):
                    g = s // GRP
                    s_loc = (s % GRP) if d == 0 else (GRP - 1 - s % GRP)
                    s_locs[d] = (G_cur[d], s_loc)
                    gt = G_cur[d]
                    rhs = Z if s == 0 else R[d][:, slot(d, s - 1), :, :]
                    for m in range(8):
                        for k in (0, 1):
                            nc.tensor.matmul(
                                gt[:, m * 64 + s_loc * NS:
                                   m * 64 + (s_loc + 1) * NS],
                                whh[:, d, k, m, :],
                                rhs[:, k, :],
                                start=False, stop=(k == 1),
                                skip_group_check=True)

                    # spread next group's ih matmuls over this group's steps
                    n_do = (len(pend[d]) + GRP - 1 - s % GRP) // (GRP - s % GRP) \
                        if pend[d] else 0
                    for _ in range(n_do):
                        pend[d].pop(0)()
                    if s % GRP == GRP - 1 and g + 2 < n_grp:
                        assert not pend[d]
                        G_nxt_new, pend[d] = make_ih_jobs(d, g + 2)
                        G_cur[d] = G_nxt[d]
                        G_nxt[d] = G_nxt_new
                    elif s % GRP == GRP - 1:
                        G_cur[d] = G_nxt[d]

                for d in (0, 1):
                    gt, s_loc = s_locs[d]
                    gview = gt[:].rearrange("p (m sb) -> p m sb", m=8)[
                        :, :, s_loc * NS:(s_loc + 1) * NS]
                    uview = U[d][:, 0:128].rearrange("p (m b) -> p m b", m=8)
                    if debug_emis and s == 0 and d == 0:
                        gg = bigp.tile([P, P], f32, name="gg")
                        nc.vector.tensor_copy(
                            gg[:].rearrange("p (m b) -> p m b", m=8), gview)
                        nc.sync.dma_start(dbgg_d[:], gg[:])
                    nc.scalar.activation(uview, gview, AF.Tanh)

                for d in (0, 1):
                    nc.vector.scalar_tensor_tensor(
                        A[d][:], U[d][:, 0:64], 1.0, U[d][:, 96:160],
                        op0=OP.add, op1=OP.mult)
                    nc.vector.scalar_tensor_tensor(
                        U[d][:, 128:160], A[d][:, 32:64], 0.5, A[d][:, 0:32],
                        op0=OP.mult, op1=OP.add)
                for d in (0, 1):
                    nc.scalar.activation(TH[d][:], U[d][:, 128:160], AF.Tanh,
                                         scale=0.5)
                for d in (0, 1):
                    nc.vector.scalar_tensor_tensor(
                        R[d][:, slot(d, s), :, :], U[d][:, 64:96], 1.0,
                        TH[d][:], op0=OP.add, op1=OP.mult)
                    if debug_emis and s == 0 and d == 0:
                        uu = bigp.tile([P, 160], f32, name="uu")
                        nc.vector.tensor_copy(uu[:], U[d][:])
                        nc.sync.dma_start(dbgu_d[:], uu[:])

                if s % ERP == ERP - 1:
                    for d in (0, 1):
                        pe = psE_p.tile([T, ERP * NS], f32, tag="psE",
                                        name=f"psE{d}_{s}")
                        for k in (0, 1):
                            nc.tensor.matmul(
                                pe[:].rearrange("p (sl b) -> p sl b", sl=ERP),
                                woutT[:, d, k, :],
                                R[d][:, :, k, :],
                                start=(k == 0), stop=(k == 1),
                                skip_group_check=True)
                        # Pool/GPSIMD cannot read PSUM on HW: copies go on DVE
                        c0 = ecol(d, s // ERP)
                        early = (c0 < SB // 2) == (d == 0)
                        if early:
                            nc.vector.tensor_copy(
                                esum[:, c0:c0 + ERP * NS], pe[:])
                        else:
                            nc.vector.tensor_tensor(
                                esum[:, c0:c0 + ERP * NS],
                                esum[:, c0:c0 + ERP * NS], pe[:], op=OP.add)

            # ---- emissions -> eexp, numerator ----
            if debug_emis:
                nc.sync.dma_start(dbg_d[:], esum[:])
                r32 = bigp.tile([P, ERP, 2, NS], f32, name="r32")
                nc.vector.tensor_copy(r32[:], R[0][:])
                nc.sync.dma_start(dbgr_d[:], r32[:])
            eexp = bigp.tile([T, SB], f32, name="eexp")
            nc.scalar.activation(eexp[:], esum[:], AF.Exp, bias=bout[:, 0:1])

            out_sb = cp.tile([1, 2 * NS], f32, name="out_sb")
            acc = cp.tile([T, NS], f32, name="acc")
            for b in range(NS):
                scr = nsc.tile([T, steps], f32, tag="numscr", name=f"nscr{b}")
                ev = esum[:].rearrange("p (s b) -> p b s", b=NS)[:, b, :]
                ov = oh[:].rearrange("p (s b) -> p b s", b=NS)[:, b, :]
                nc.vector.scalar_tensor_tensor(
                    scr[:], ev, 1.0, ov, op0=OP.mult, op1=OP.mult,
                    accum_out=acc[:, b:b + 1])
            pnt = psE_p.tile([T, ERP * NS], f32, tag="psE", name="pnumt")
            pnum = pnt[0:1, 0:NS]
            nc.tensor.matmul(pnum, ones50c[:], acc[:], start=True, stop=True,
                             skip_group_check=True)
            nc.vector.tensor_copy(out_sb[0:1, 0:NS], pnum)

            # ---- CRF partition function: alpha-forward and beta-backward
            # halves run concurrently and meet in the middle:
            #   Z = sum_i alpha_K[i] * beta_K[i],  K = steps//2 - 1.
            # One full-width [50,16] multiply per chain-step keeps DVE
            # fixed costs down; rescale sums are stashed and ln()'d in one
            # deferred batched pass.
            RESC = 16
            KMID = steps // 2 - 1
            n_resc = steps // RESC + 2
            shist = [cp.tile([1, n_resc * NS], f32, name=f"shist{c}")
                     for c in (0, 1)]
            nresc_done = [0, 0]

            def rescale(c, cur, j):
                pst = psC.tile([T, NS], f32, tag="pa", name=f"pss{c}_{j}")
                ps_s = pst[0:1, :]
                nc.tensor.matmul(ps_s, ones50c[:], cur[:],
                                 start=True, stop=True, skip_group_check=True)
                rinv = sc.tile([1, NS], f32, tag=f"ri{c}", name=f"ri{c}_{j}")
                nc.vector.reciprocal(rinv[:], ps_s)
                ri = nresc_done[c]
                nc.vector.tensor_copy(shist[c][:, ri * NS:(ri + 1) * NS], ps_s)
                nresc_done[c] += 1
                pb = psC.tile([T, NS], f32, tag="pa", name=f"pb{c}_{j}")
                nc.tensor.matmul(pb[:], ones50r[:], rinv[:],
                                 start=True, stop=True, skip_group_check=True)
                ar = sc.tile([T, NS], f32, tag=f"a{c}", name=f"ar{c}_{j}")
                nc.vector.tensor_tensor(ar[:], pb[:], cur[:], op=OP.mult)
                return ar

            # alpha chain state (c=0), beta chain state y = e_k * beta_k (c=1)
            a_cur = sc.tile([T, NS], f32, tag="a0", name="alf0")
            nc.vector.tensor_scalar(
                out=a_cur[:], in0=eexp[:, 0:NS],
                scalar1=estart[:, 0:1], scalar2=None, op0=OP.mult)
            y_cur = sc.tile([T, NS], f32, tag="a1", name="bet0")
            nc.vector.tensor_scalar(
                out=y_cur[:], in0=eexp[:, (steps - 1) * NS:steps * NS],
                scalar1=eend[:, 0:1], scalar2=None, op0=OP.mult)

            ka = list(range(1, KMID + 1))          # alpha: absorb e_k
            kb = list(range(steps - 2, KMID, -1))  # beta: MM then absorb e_k
            b_fin = None
            for j in range(max(len(ka), len(kb) + 1)):
                if j < len(ka):
                    k = ka[j]
                    pa = psC.tile([T, NS], f32, tag="pa", name=f"paA_{k}")
                    nc.tensor.matmul(pa[:], etr[:], a_cur[:],
                                     start=True, stop=True,
                                     skip_group_check=True)
                    an = sc.tile([T, NS], f32, tag="a0", name=f"alf{k}")
                    nc.vector.tensor_tensor(
                        an[:], pa[:], eexp[:, k * NS:(k + 1) * NS],
                        op=OP.mult)
                    a_cur = an
                    if j % RESC == 14:
                        a_cur = rescale(0, a_cur, j)
                if j < len(kb):
                    k = kb[j]
                    pb_ = psC.tile([T, NS], f32, tag="pa", name=f"pbB_{k}")
                    nc.tensor.matmul(pb_[:], etrB[:], y_cur[:],
                                     start=True, stop=True,
                                     skip_group_check=True)
                    yn = sc.tile([T, NS], f32, tag="a1", name=f"bet{k}")
                    nc.vector.tensor_tensor(
                        yn[:], pb_[:], eexp[:, k * NS:(k + 1) * NS],
                        op=OP.mult)
                    y_cur = yn
                    if j % RESC == 6:
                        y_cur = rescale(1, y_cur, j)
                elif j == len(kb):
                    # last beta MM: beta_KMID = etr @ y_{KMID+1}
                    bf = psC.tile([T, NS], f32, tag="pa", name="betfin")
                    nc.tensor.matmul(bf[:], etrB[:], y_cur[:],
                                     start=True, stop=True,
                                     skip_group_check=True)
                    b_fin = bf

            w = sc.tile([T, NS], f32, tag="a0", name="w_ab")
            nc.vector.tensor_tensor(w[:], a_cur[:], b_fin[:], op=OP.mult)
            pzt = psC.tile([T, NS], f32, tag="pa", name="pzf")
            pz = pzt[0:1, :]
            nc.tensor.matmul(pz, ones50c[:], w[:],
                             start=True, stop=True, skip_group_check=True)
            lz = sc.tile([1, NS], f32, tag="lz", name="lzf")
            _ln_any(nc, sc, lz[:], pz, NS, "fin")
            lsum = sc.tile([1, NS], f32, tag="lsum", name="lsum")
            nc.vector.tensor_copy(lsum[:], lz[:])
            for c in (0, 1):
                nr = nresc_done[c]
                lnh = sc.tile([1, nr * NS], f32, tag="lnh", name=f"lnh{c}")
                _ln_any(nc, sc, lnh[:], shist[c][:, 0:nr * NS], nr * NS,
                        f"hist{c}")
                lacc = sc.tile([1, NS, 1], f32, tag="laccr", name=f"laccr{c}")
                nc.vector.tensor_reduce(
                    lacc[:],
                    lnh[:].rearrange("o (r b) -> o b r", b=NS),
                    axis=mybir.AxisListType.X, op=OP.add)
                nc.vector.tensor_tensor(lsum[:], lsum[:], lacc[:, :, 0],
                                        op=OP.add)
            nc.vector.tensor_copy(out_sb[0:1, NS:2 * NS], lsum[:])

            nc.sync.dma_start(out_d[:], out_sb[:])
    return nc


# ---------------------------------------------------------------------------
# Host-side preparation
# ---------------------------------------------------------------------------
# device gate-chunk order: i0 i1 f0 f1 o0 o1 g0 g1 (torch rows i,f,g,o)
_M_ROWS = [0, 1, 2, 3, 6, 7, 4, 5]   # torch 128-row chunk for device chunk m
_M_SCALE = [.5, .5, .5, .5, .5, .5, 1., 1.]


def _prep_dir_weights(w_ih, w_hh, b_ih, b_hh, w_out_half):
    """Returns (wih [128,2,8,128], whh [128,2,8,128], bias [1024],
    wout [128,2,50]) with all scalings applied."""
    wih_c = np.zeros((P, 2, 8, P), dtype=bfnp)
    whh_c = np.zeros((P, 2, 8, P), dtype=bfnp)
    bias = np.zeros(1024, dtype=np.float32)
    bsum = b_ih + b_hh
    for m in range(8):
        r0 = _M_ROWS[m] * P
        sc_m = _M_SCALE[m]
        for k in range(2):
            wih_c[:, k, m, :] = (w_ih[r0:r0 + P, k * P:(k + 1) * P].T
                                 * sc_m).astype(bfnp)
            # extra 0.5: the device h state carries 2h
            whh_c[:, k, m, :] = (w_hh[r0:r0 + P, k * P:(k + 1) * P].T
                                 * sc_m * 0.5).astype(bfnp)
        bias[m * P:(m + 1) * P] = bsum[r0:r0 + P] * sc_m
    wout_c = np.zeros((P, 2, T), dtype=bfnp)
    for k in range(2):
        wout_c[:, k, :] = (w_out_half[:, k * P:(k + 1) * P].T * 0.5).astype(bfnp)
    return wih_c, whh_c, bias.astype(bfnp), wout_c


def prep_inputs(inputs, steps=S):
    sent = np.asarray(inputs["sentences"])[:, :steps]
    tags = np.asarray(inputs["tags"])[:, :steps]
    emb = np.asarray(inputs["embedding"], dtype=np.float32)
    W_out = np.asarray(inputs["W_out"], dtype=np.float32)

    wihs, whhs, biases, wouts = [], [], [], []
    for d, sfx in enumerate("fb"):
        wi, wh, bs, wo = _prep_dir_weights(
            np.asarray(inputs[f"w_ih_{sfx}"], dtype=np.float32),
            np.asarray(inputs[f"w_hh_{sfx}"], dtype=np.float32),
            np.asarray(inputs[f"b_ih_{sfx}"], dtype=np.float32),
            np.asarray(inputs[f"b_hh_{sfx}"], dtype=np.float32),
            W_out[:, d * H:(d + 1) * H])
        wihs.append(wi); whhs.append(wh); biases.append(bs); wouts.append(wo)
    wih_all = np.ascontiguousarray(np.stack(wihs, axis=1))      # [128,2,2,8,128]
    whh_all = np.ascontiguousarray(np.stack(whhs, axis=1))
    bias_all = np.stack(biases, axis=0)[None]                    # [1,2,1024]
    wout_all = np.ascontiguousarray(np.stack(wouts, axis=1))     # [128,2,2,50]

    trans = np.asarray(inputs["trans"], dtype=np.float32)
    etrans = np.exp(trans).astype(np.float32)
    estart = np.exp(np.asarray(inputs["start_trans"],
                               dtype=np.float32)).reshape(T, 1)
    eend = np.exp(np.asarray(inputs["end_trans"],
                             dtype=np.float32)).reshape(T, 1)
    b_out = np.asarray(inputs["b_out"], dtype=np.float32).reshape(T, 1)

    xs = emb[sent].astype(bfnp)                     # [B, steps, E]
    in_maps = []
    for core in range(NCORES):
        b0 = core * NS
        xc = xs[b0:b0 + NS]                          # [NS, steps, E]
        # -> [E(k,p), steps*NS] with col = s*NS + b
        xT = np.ascontiguousarray(
            xc.transpose(2, 1, 0).reshape(2, P, steps * NS)
            .transpose(1, 0, 2))
        tc_ = tags[b0:b0 + NS]                       # [NS, steps]
        ohc = (np.arange(T)[:, None, None]
               == tc_.T[None, :, :]).astype(bfnp)    # [T, steps, NS]
        in_maps.append({
            "xT": xT,
            "wih": wih_all, "whh": whh_all, "biasT": bias_all,
            "woutT": wout_all,
            "etrans": etrans, "etransB": np.ascontiguousarray(etrans.T), "bout": b_out, "estart": estart, "eend": eend,
            "oh": np.ascontiguousarray(ohc.reshape(T, steps * NS)),
        })
    return in_maps


def host_numerator_part(inputs, steps=S):
    tags = np.asarray(inputs["tags"])[:, :steps]
    trans = np.asarray(inputs["trans"], dtype=np.float32)
    b_out = np.asarray(inputs["b_out"], dtype=np.float32)
    start_trans = np.asarray(inputs["start_trans"], dtype=np.float32)
    end_trans = np.asarray(inputs["end_trans"], dtype=np.float32)
    num = start_trans[tags[:, 0]] + end_trans[tags[:, -1]]
    num = num + trans[tags[:, :-1], tags[:, 1:]].sum(axis=1)
    num = num + b_out[tags].sum(axis=1)
    return num.astype(np.float32)


_cache = {}


def _get_nc(steps):
    if steps not in _cache:
        _cache[steps] = build_fused(steps)
    return _cache[steps]


def run_fused(inputs, steps=S, trace=False):
    nc = _get_nc(steps)
    in_maps = prep_inputs(inputs, steps)
    res = run_bass_kernel_spmd(nc, in_maps, core_ids=list(range(NCORES)),
                               trace=trace)
    num_e = np.concatenate([r["out"][0, 0:NS] for r in res.results])
    logz = np.concatenate([r["out"][0, NS:2 * NS] for r in res.results])
    num_h = host_numerator_part(inputs, steps)
    loss = -np.mean(num_h + num_e - logz)
    return np.float32(loss), {
        "num_e": num_e, "logz": logz, "num_h": num_h,
        "exec": res.exec_time_ns,
    }


def kernel(**inputs):
    loss, _ = run_fused(inputs, steps=S, trace=False)
    return np.asarray(loss, dtype=np.float32)
